# revision 11
# baseline (speedup 1.0000x reference)
"""Trainium2 Bass kernel for the heap-ancestor MLP (gnn_message_passing).

Math: heap_linear(x, W, b, minheap) reduces to, for output node j:
    out[j] = sum_{k=minheap}^{min(L(j),12)} ( W_k @ x[j >> k] + b_k )
where L(j) is the heap level of j (L(0)=0, else floor(log2 j)+1) and the
step-k mask is simply j >= 2^(k-1).  The gather j>>k over a 512-aligned
tile of consecutive j is a repeat-interleave of a contiguous slice, which
the PE's moving-operand access pattern expresses directly with a step-0
inner dim (zero copy).  All k-steps accumulate in PSUM; the masked bias
sum collapses to a per-level cumulative bias applied with one extra K=13
matmul against a constant level-one-hot matrix.

Sharding: pure data parallelism over batch (256 -> 32 per core x 8 cores),
weights + constants replicated.
"""

import os

import numpy as np

import concourse.bass as bass
from concourse import bacc
import concourse.mybir as mybir
import concourse.tile as tile
from concourse.bass_utils import run_bass_kernel_spmd

UNITS = 4096
DEPTH = 13
JT = 512                     # j-tile size
NT = UNITS // JT             # 8 j-tiles
NCORES = 8
BATCH = 256
NB = BATCH // NCORES         # 32 batch elements per core

F32 = mybir.dt.float32

# mode: "f32" (exact, 4 cyc/col), "f32r" (1 cyc/col, reduced precision),
# "bf16" (1 cyc/col, bf16 storage)
MODE = os.environ.get("KER_MODE", "f32r")


def _levels() -> np.ndarray:
    lv = np.zeros(UNITS, np.int64)
    for j in range(1, UNITS):
        lv[j] = int(np.floor(np.log2(j))) + 1
    return lv


def _kmax(t: int) -> int:
    # max valid heap step k for j-tile t (tiles >= 1 are level-uniform)
    if t == 0:
        return 9
    return int(np.floor(np.log2(JT * t))) + 1


def _bcast(ap, n_dist: int, rep: int):
    """[P, n_dist] slice -> [P, n_dist, rep] repeat-interleave AP."""
    return ap.unsqueeze(2).broadcast_to([ap.shape[0], n_dist, rep])


# packed constant layout: one [64, _CW] tensor, one DMA, one semaphore
_CREG = {}
_c = 0
for _name, _w in [("w2t", DEPTH * 64), ("w3t", DEPTH * 2), ("w1t", 12 * 64),
                  ("oh", UNITS), ("bc1", 64), ("bc2", 64), ("bc3", 32)]:
    _CREG[_name] = (_c, _c + _w)
    _c += _w
_CW = _c


def _emit_heap_group(nc, md, ps, group, h, w_of_k, bc_ap, oh_sb, m, kmin):
    """Accumulate sum_{k>=kmin} W_k h[:, j>>k] + levelbias into
    ps[po:po+m, :] for each (po, t) in group, interleaving the groups'
    matmuls per k so different col-group tiles overlap in the PE array.

    h: [K, UNITS] source AP; w_of_k(k): [K, m] lhsT slice; bc_ap: [13, mp]
    cumulative level bias (mp = max(m, 32) so start=True initializes the
    full 32-partition group when m == 2); oh_sb: [13, UNITS] level one-hot.
    """
    mp = 32 if m == 2 else m
    skip = True  # sim group checker mis-maps partition-sliced psum tiles
    P = h.shape[0]
    for po, t in group:
        nc.tensor.matmul(
            ps[po : po + mp, :],
            md(bc_ap),
            md(oh_sb[:, t * JT : (t + 1) * JT]),
            start=True,
            stop=False,
            tile_position=(0, po),
            skip_group_check=skip,
        )
    kmaxes = {t: _kmax(t) for _, t in group}
    for k in range(kmin, max(kmaxes.values()) + 1):
        w = md(w_of_k(k))
        for po, t in group:
            kmax = kmaxes[t]
            if k > kmax:
                continue
            c0 = t * JT
            om = ps[po : po + m, :]
            last = k == kmax
            if k == 0:
                nc.tensor.matmul(
                    om,
                    w,
                    md(h[:, c0 : c0 + JT]),
                    start=False,
                    stop=last,
                    tile_position=(0, po),
                    skip_group_check=skip,
                )
            elif t == 0:
                # tile 0: partial columns.  cols [2^(k-1), 2^k): ancestor
                # 0; cols [2^k, JT): ancestors [1, JT>>k) repeated 2^k.
                half = 1 << (k - 1)
                blk = 1 << k
                nc.tensor.matmul(
                    ps[po : po + m, half:blk],
                    w,
                    md(h[:, 0:1].broadcast_to([P, half])),
                    start=False,
                    stop=last and blk >= JT,
                    tile_position=(0, po),
                    skip_group_check=skip,
                )
                if blk < JT:
                    nd = (JT >> k) - 1
                    nc.tensor.matmul(
                        ps[po : po + m, blk:JT],
                        w,
                        md(_bcast(h[:, 1 : 1 + nd], nd, blk)),
                        start=False,
                        stop=last,
                        tile_position=(0, po),
                        skip_group_check=skip,
                    )
            elif (JT >> k) >= 1:
                nd = JT >> k
                a0 = c0 >> k
                nc.tensor.matmul(
                    om,
                    w,
                    md(_bcast(h[:, a0 : a0 + nd], nd, 1 << k)),
                    start=False,
                    stop=last,
                    tile_position=(0, po),
                    skip_group_check=skip,
                )
            else:
                a = c0 >> k
                nc.tensor.matmul(
                    om,
                    w,
                    md(h[:, a : a + 1].broadcast_to([P, JT])),
                    start=False,
                    stop=last,
                    tile_position=(0, po),
                    skip_group_check=skip,
                )


def build_program(nb: int, mode: str) -> bass.Bass:
    sdt = mybir.dt.bfloat16 if mode == "bf16" else F32

    def md(ap):
        return ap.bitcast(mybir.dt.float32r) if mode == "f32r" else ap

    nc = bacc.Bacc()
    xt = nc.declare_dram_parameter("xt", [nb, 2, UNITS], sdt, isOutput=False)
    cst = nc.declare_dram_parameter("cst", [64, _CW], sdt, isOutput=False)
    out = nc.declare_dram_parameter("out", [nb, NT, 2, JT], F32, isOutput=True)

    relu = mybir.ActivationFunctionType.Relu

    with tile.TileContext(nc) as tc:
        with (
            tc.tile_pool(name="const", bufs=1) as cp,
            tc.tile_pool(name="xb", bufs=3) as xp,
            tc.tile_pool(name="h1", bufs=2) as h1p,
            tc.tile_pool(name="h2", bufs=2) as h2p,
            tc.tile_pool(name="st", bufs=3) as stp,
            tc.tile_pool(name="ps", bufs=6, space="PSUM") as psp,
        ):
            cst_sb = cp.tile([64, _CW], sdt)
            nc.sync.dma_start(out=cst_sb[:], in_=cst[:, :])

            def _reg(name, rows):
                lo, hi = _CREG[name]
                return cst_sb[0:rows, lo:hi]

            zz_sb = cp.tile([64, 1], F32)
            nc.vector.memset(zz_sb[:], 0.0)

            w1t_sb = _reg("w1t", 2)
            w2t_sb = _reg("w2t", 64)
            w3t_sb = _reg("w3t", 64)
            bc1_sb = _reg("bc1", DEPTH)
            bc2_sb = _reg("bc2", DEPTH)
            bc3_sb = _reg("bc3", DEPTH)
            oh_sb = _reg("oh", DEPTH)

            for b in range(nb):
                x_sb = xp.tile([2, UNITS], sdt)
                nc.sync.dma_start(out=x_sb[:], in_=xt[b, :, :])

                # ---- layer 1: h1 = relu(sum_{k>=1} W1_k x[j>>k] + bias) ----
                h1 = h1p.tile([64, UNITS], sdt)
                for p in range(NT // 2):
                    ps = psp.tile([128, JT], F32)
                    _emit_heap_group(
                        nc, md, ps, [(0, 2 * p), (64, 2 * p + 1)], x_sb[:],
                        lambda k: w1t_sb[:, 64 * (k - 1) : 64 * k],
                        bc1_sb, oh_sb, 64, kmin=1,
                    )
                    for half in range(2):
                        t = 2 * p + half
                        po = 64 * half
                        cs = slice(t * JT, (t + 1) * JT)
                        nc.scalar.activation(h1[:, cs], ps[po : po + 64, :], relu)

                # ---- layer 2: h2 = relu(sum_k W2_k h1[j>>k] + bias) ----
                h2 = h2p.tile([64, UNITS], sdt)
                for p in range(NT // 2):
                    ps = psp.tile([128, JT], F32)
                    _emit_heap_group(
                        nc, md, ps, [(0, 2 * p), (64, 2 * p + 1)], h1[:],
                        lambda k: w2t_sb[:, 64 * k : 64 * (k + 1)],
                        bc2_sb, oh_sb, 64, kmin=0,
                    )
                    for half in range(2):
                        t = 2 * p + half
                        po = 64 * half
                        cs = slice(t * JT, (t + 1) * JT)
                        nc.vector.tensor_scalar_max(h2[:, cs], ps[po : po + 64, :], zz_sb[:, 0:1])

                # ---- layer 3: logits (no relu), 4 j-tiles per PSUM bank ----
                for pp in range(2):
                    ps = psp.tile([128, JT], F32)
                    _emit_heap_group(
                        nc, md, ps,
                        [(32 * c, 4 * pp + c) for c in range(4)], h2[:],
                        lambda k: w3t_sb[:, 2 * k : 2 * (k + 1)],
                        bc3_sb, oh_sb, 2, kmin=0,
                    )
                    st = stp.tile([128, JT], F32)
                    nc.scalar.activation(
                        st[:], ps[:], mybir.ActivationFunctionType.Copy
                    )
                    for c in range(4):
                        t = 4 * pp + c
                        po = 32 * c
                        nc.sync.dma_start(
                            out=out[b, t, :, :], in_=st[po : po + 2, :]
                        )
    nc.compile()
    return nc


def _prep_host(x, W1, b1, W2, b2, W3, b3, mode: str):
    lv = _levels()
    npdt = np.dtype(mybir.dt.np(mybir.dt.bfloat16)) if mode == "bf16" else np.float32

    oh = np.zeros((DEPTH, UNITS), np.float32)
    oh[lv, np.arange(UNITS)] = 1.0

    # cumulative (masked-sum) biases per level
    bc1 = np.zeros((DEPTH, 64), np.float32)
    for L in range(DEPTH):
        if L >= 1:
            bc1[L] = b1[: min(L, 12)].sum(0)
    bc2 = np.zeros((DEPTH, 64), np.float32)
    for L in range(DEPTH):
        bc2[L] = b2[: L + 1].sum(0)
    bc3 = np.zeros((DEPTH, 32), np.float32)
    for L in range(DEPTH):
        bc3[L, :2] = b3[: L + 1].sum(0)

    w1t = W1.transpose(2, 0, 1).reshape(2, 12 * 64)
    w2t = W2.transpose(2, 0, 1).reshape(64, DEPTH * 64)
    w3t = W3.transpose(2, 0, 1).reshape(64, DEPTH * 2)

    cstm = np.zeros((64, _CW), np.float32)
    for name, arr in [("w2t", w2t), ("w3t", w3t), ("w1t", w1t), ("oh", oh),
                      ("bc1", bc1), ("bc2", bc2), ("bc3", bc3)]:
        lo, hi = _CREG[name]
        cstm[: arr.shape[0], lo:hi] = arr
    common = {"cst": cstm.astype(npdt)}
    nb = x.shape[0] // NCORES
    in_maps = []
    for c in range(NCORES):
        xs = x[c * nb : (c + 1) * nb].transpose(0, 2, 1)  # [nb, 2, UNITS]
        m = dict(common)
        m["xt"] = np.ascontiguousarray(xs).astype(npdt)
        in_maps.append(m)
    return in_maps


def kernel(x, W1, b1, W2, b2, W3, b3, _trace=False, _mode=None):
    mode = _mode or MODE
    nb = x.shape[0] // NCORES
    nc = build_program(nb, mode)
    in_maps = _prep_host(x, W1, b1, W2, b2, W3, b3, mode)
    res = run_bass_kernel_spmd(
        nc, in_maps, core_ids=list(range(NCORES)), trace=_trace
    )
    outs = []
    for r in res.results:
        o = r["out"]  # [nb, NT, 2, JT]
        outs.append(o.transpose(0, 1, 3, 2).reshape(nb, UNITS, 2))
    full = np.concatenate(outs, 0).astype(np.float32)
    if _trace:
        kernel.last_exec_time_ns = res.exec_time_ns
        kernel.last_results = res
    return full


kernel.last_exec_time_ns = None
kernel.last_results = None


# revision 16
# speedup vs baseline: 8.9697x; 8.9697x over previous
"""Trainium2 Bass kernel for the heap-ancestor MLP (gnn_message_passing).

Math: heap_linear(x, W, b, minheap) reduces to, for output node j:
    out[j] = sum_{k=minheap}^{min(L(j),12)} ( W_k @ x[j >> k] + b_k )
where L(j) is the heap level of j (L(0)=0, else floor(log2 j)+1) and the
step-k mask is simply j >= 2^(k-1).  The gather j>>k over a 512-aligned
tile of consecutive j is a repeat-interleave of a contiguous slice, which
the PE's moving-operand access pattern expresses directly with a step-0
inner dim (zero copy).  All k-steps accumulate in PSUM; the masked bias
sum collapses to a per-level cumulative bias applied with one extra K=13
matmul against a constant level-one-hot matrix.

Throughput: the PE moving-operand stream is ~1 column/cycle per source
partition group.  Four j-tiles run concurrently as a "quad" — tile i at
(row_group, col_pos) in {(0,0),(0,64),(64,0),(64,64)}, each accumulating
in its own PSUM bank (row tiles must not share a bank), with activations
duplicated to SBUF partitions 64-127 so both row groups can stream.

Sharding: pure data parallelism over batch (256 -> 32 per core x 8 cores),
weights + constants replicated.
"""

import os

import numpy as np

import concourse.bass as bass
from concourse import bacc
import concourse.mybir as mybir
import concourse.tile as tile
from concourse.bass_utils import run_bass_kernel_spmd

UNITS = 4096
DEPTH = 13
JT = 512                     # j-tile size
NT = UNITS // JT             # 8 j-tiles
NCORES = 8
BATCH = 256
NB = BATCH // NCORES         # 32 batch elements per core

F32 = mybir.dt.float32

# mode: "f32" (exact, 4 cyc/col), "f32r" (1 cyc/col, reduced precision),
# "bf16" (1 cyc/col, bf16 storage)
MODE = os.environ.get("KER_MODE", "bf16")

# (row_group, col_pos) per tile slot within a quad
QPOS = [(0, 0), (0, 64), (64, 0), (64, 64)]

# packed constant layout: one [128, _CW] tensor (rows 64-127 duplicate
# rows 0-63 so row-group-64 matmuls can read weights), one DMA
_CREG = {}
_c = 0
for _name, _w in [("w2t", DEPTH * 64), ("w3t", DEPTH * 2), ("w1t", 12 * 64),
                  ("oh", UNITS), ("bc1", 64), ("bc2", 64), ("bc3", 32)]:
    _CREG[_name] = (_c, _c + _w)
    _c += _w
_CW = _c


def _levels() -> np.ndarray:
    lv = np.zeros(UNITS, np.int64)
    for j in range(1, UNITS):
        lv[j] = int(np.floor(np.log2(j))) + 1
    return lv


def _kmax(t: int) -> int:
    # max valid heap step k for j-tile t (tiles >= 1 are level-uniform)
    if t == 0:
        return 9
    return int(np.floor(np.log2(JT * t))) + 1


def _bcast(ap, n_dist: int, rep: int):
    """[P, n_dist] slice -> [P, n_dist, rep] repeat-interleave AP."""
    return ap.unsqueeze(2).broadcast_to([ap.shape[0], n_dist, rep])


def _emit_heap_quad(nc, md, quad, h, w_of, bc_of, oh_of, m, kmin, kw):
    """Emit the heap-step matmuls for up to 4 j-tiles (a quad).

    quad: list of (ps_ap, r, c, t) — psum pool tile (own bank), row group,
      col position, j-tile index.  Matmuls for the tiles are interleaved
      per k so the row groups stream concurrently.
    h: [128, UNITS] source (rows 64-127 duplicate 0-63); w_of(r, k): [K, m]
      lhsT slice at partition base r; bc_of(r): [13, mp] bias lhsT at base
      r; oh_of(r): [13, UNITS] one-hot at base r.  kw: source K width.
    """
    mp = 32 if m == 2 else m
    for ps, r, c, t in quad:
        nc.tensor.matmul(
            ps[c : c + mp, :],
            md(bc_of(r)),
            md(oh_of(r)[:, t * JT : (t + 1) * JT]),
            start=True,
            stop=False,
            tile_position=(r, c),
            skip_group_check=True,
        )
    kmaxes = [_kmax(t) for _, _, _, t in quad]
    for k in range(kmin, max(kmaxes) + 1):
        for (ps, r, c, t), kmax in zip(quad, kmaxes):
            if k > kmax:
                continue
            c0 = t * JT
            hs = h[r : r + kw, :]
            om = ps[c : c + m, :]
            last = k == kmax
            if k == 0:
                nc.tensor.matmul(
                    om,
                    md(w_of(r, k)),
                    md(hs[:, c0 : c0 + JT]),
                    start=False,
                    stop=last,
                    tile_position=(r, c),
                    skip_group_check=True,
                )
            elif t == 0:
                # tile 0: partial columns.  cols [2^(k-1), 2^k): ancestor
                # 0; cols [2^k, JT): ancestors [1, JT>>k) repeated 2^k.
                half = 1 << (k - 1)
                blk = 1 << k
                nc.tensor.matmul(
                    ps[c : c + m, half:blk],
                    md(w_of(r, k), half),
                    md(hs[:, 0:1].broadcast_to([kw, half]), half),
                    start=False,
                    stop=last and blk >= JT,
                    tile_position=(r, c),
                    skip_group_check=True,
                )
                if blk < JT:
                    nd = (JT >> k) - 1
                    nc.tensor.matmul(
                        ps[c : c + m, blk:JT],
                        md(w_of(r, k), JT - blk),
                        md(_bcast(hs[:, 1 : 1 + nd], nd, blk), JT - blk),
                        start=False,
                        stop=last,
                        tile_position=(r, c),
                        skip_group_check=True,
                    )
            elif (JT >> k) >= 1:
                nd = JT >> k
                a0 = c0 >> k
                nc.tensor.matmul(
                    om,
                    md(w_of(r, k)),
                    md(_bcast(hs[:, a0 : a0 + nd], nd, 1 << k)),
                    start=False,
                    stop=last,
                    tile_position=(r, c),
                    skip_group_check=True,
                )
            else:
                a = c0 >> k
                nc.tensor.matmul(
                    om,
                    md(w_of(r, k)),
                    md(hs[:, a : a + 1].broadcast_to([kw, JT])),
                    start=False,
                    stop=last,
                    tile_position=(r, c),
                    skip_group_check=True,
                )


def _dup_dmas(nc, h, qt):
    """Duplicate the freshly-written halves of h for quad qt (tiles 4qt..
    4qt+3): tiles with c=0 wrote rows 0-63 (copy up), c=64 wrote rows
    64-127 (copy down).  One DMA per 512-block."""
    base = qt * 4 * JT
    for c_src in (0, 64):
        dst = 64 - c_src
        for blk in range(2):
            o = base + blk * 2 * JT + (JT if c_src == 64 else 0)
            nc.sync.dma_start(
                out=h[dst : dst + 64, o : o + JT],
                in_=h[c_src : c_src + 64, o : o + JT],
            )


def build_program(nb: int, mode: str) -> bass.Bass:
    sdt = {"bf16": mybir.dt.bfloat16, "f32r": mybir.dt.float32r, "f32": F32}[mode]

    def md(ap, n=JT):
        if mode == "f32r" and n < 256:
            return ap.bitcast(F32)
        return ap

    nc = bacc.Bacc()
    xt = nc.declare_dram_parameter("xt", [nb, 2, UNITS], sdt, isOutput=False)
    cst = nc.declare_dram_parameter("cst", [128, _CW], sdt, isOutput=False)
    out = nc.declare_dram_parameter("out", [nb, NT, 2, JT], F32, isOutput=True)

    relu = mybir.ActivationFunctionType.Relu

    with tile.TileContext(nc) as tc:
        with (
            tc.tile_pool(name="const", bufs=1) as cp,
            tc.tile_pool(name="xb", bufs=3) as xp,
            tc.tile_pool(name="h1", bufs=2) as h1p,
            tc.tile_pool(name="h2", bufs=2) as h2p,
            tc.tile_pool(name="st", bufs=4) as stp,
            tc.tile_pool(name="ps", bufs=8, space="PSUM") as psp,
        ):
            cst_sb = cp.tile([128, _CW], sdt)
            nc.sync.dma_start(out=cst_sb[:], in_=cst[:, :])
            zz_sb = cp.tile([128, 1], F32)
            nc.vector.memset(zz_sb[:], 0.0)

            def _reg(name, rows):
                lo, hi = _CREG[name]

                def f(r):
                    return cst_sb[r : r + rows, lo:hi]

                return f

            w1t = _reg("w1t", 2)
            w2t = _reg("w2t", 64)
            w3t = _reg("w3t", 64)
            bc1 = _reg("bc1", DEPTH)
            bc2 = _reg("bc2", DEPTH)
            bc3 = _reg("bc3", DEPTH)
            oh = _reg("oh", DEPTH)

            for b in range(nb):
                x_sb = xp.tile([128, UNITS], sdt)
                nc.sync.dma_start(out=x_sb[0:2, :], in_=xt[b, :, :])
                nc.sync.dma_start(out=x_sb[64:66, :], in_=xt[b, :, :])

                # ---- layer 1: h1 = relu(sum_{k>=1} W1_k x[j>>k] + bias) ----
                h1 = h1p.tile([128, UNITS], sdt)
                for qt in range(2):
                    quad = []
                    for i, (r, c) in enumerate(QPOS):
                        pq = psp.tile([128, JT], F32, tag="ps")
                        quad.append((pq, r, c, 4 * qt + i))
                    _emit_heap_quad(
                        nc, md, quad, x_sb[:],
                        lambda r, k: w1t(r)[:, 64 * (k - 1) : 64 * k],
                        bc1, oh, 64, 1, 2,
                    )
                    for ps, r, c, t in quad:
                        cs = slice(t * JT, (t + 1) * JT)
                        nc.scalar.activation(
                            h1[c : c + 64, cs], ps[c : c + 64, :], relu
                        )
                    _dup_dmas(nc, h1, qt)

                # ---- layer 2: h2 = relu(sum_k W2_k h1[j>>k] + bias) ----
                h2 = h2p.tile([128, UNITS], sdt)
                for qt in range(2):
                    quad = []
                    for i, (r, c) in enumerate(QPOS):
                        pq = psp.tile([128, JT], F32, tag="ps")
                        quad.append((pq, r, c, 4 * qt + i))
                    _emit_heap_quad(
                        nc, md, quad, h1[:],
                        lambda r, k: w2t(r)[:, 64 * k : 64 * (k + 1)],
                        bc2, oh, 64, 0, 64,
                    )
                    for ps, r, c, t in quad:
                        cs = slice(t * JT, (t + 1) * JT)
                        nc.vector.tensor_scalar_max(
                            h2[c : c + 64, cs], ps[c : c + 64, :],
                            zz_sb[c : c + 64, 0:1],
                        )
                    _dup_dmas(nc, h2, qt)

                # ---- layer 3: logits (no relu) ----
                for qt in range(2):
                    quad = []
                    for i, (r, cc) in enumerate(
                        [(0, 0), (0, 32), (64, 64), (64, 96)]
                    ):
                        pq = psp.tile([128, JT], F32, tag="ps")
                        quad.append((pq, r, cc, 4 * qt + i))
                    _emit_heap_quad(
                        nc, md, quad, h2[:],
                        lambda r, k: w3t(r)[:, 2 * k : 2 * (k + 1)],
                        bc3, oh, 2, 0, 64,
                    )
                    st = stp.tile([128, JT], F32)
                    for idx, (ps, r, c, t) in enumerate(quad):
                        if idx % 2 == 0:
                            nc.scalar.activation(
                                st[c : c + 32, :], ps[c : c + 32, :],
                                mybir.ActivationFunctionType.Copy,
                            )
                        else:
                            nc.vector.tensor_copy(
                                st[c : c + 32, :], ps[c : c + 32, :]
                            )
                    for ps, r, c, t in quad:
                        nc.sync.dma_start(
                            out=out[b, t, :, :], in_=st[c : c + 2, :]
                        )
    nc.compile()
    return nc


def _prep_host(x, W1, b1, W2, b2, W3, b3, mode: str):
    lv = _levels()
    npdt = np.dtype(mybir.dt.np(mybir.dt.bfloat16)) if mode == "bf16" else np.float32

    oh = np.zeros((DEPTH, UNITS), np.float32)
    oh[lv, np.arange(UNITS)] = 1.0

    # cumulative (masked-sum) biases per level
    bc1 = np.zeros((DEPTH, 64), np.float32)
    for L in range(DEPTH):
        if L >= 1:
            bc1[L] = b1[: min(L, 12)].sum(0)
    bc2 = np.zeros((DEPTH, 64), np.float32)
    for L in range(DEPTH):
        bc2[L] = b2[: L + 1].sum(0)
    bc3 = np.zeros((DEPTH, 32), np.float32)
    for L in range(DEPTH):
        bc3[L, :2] = b3[: L + 1].sum(0)

    w1t = W1.transpose(2, 0, 1).reshape(2, 12 * 64)
    w2t = W2.transpose(2, 0, 1).reshape(64, DEPTH * 64)
    w3t = W3.transpose(2, 0, 1).reshape(64, DEPTH * 2)

    cstm = np.zeros((128, _CW), np.float32)
    for name, arr in [("w2t", w2t), ("w3t", w3t), ("w1t", w1t), ("oh", oh),
                      ("bc1", bc1), ("bc2", bc2), ("bc3", bc3)]:
        lo, hi = _CREG[name]
        cstm[: arr.shape[0], lo:hi] = arr
        cstm[64 : 64 + arr.shape[0], lo:hi] = arr
    common = {"cst": cstm.astype(npdt)}
    nb = x.shape[0] // NCORES
    in_maps = []
    for c in range(NCORES):
        xs = x[c * nb : (c + 1) * nb].transpose(0, 2, 1)  # [nb, 2, UNITS]
        m = dict(common)
        m["xt"] = np.ascontiguousarray(xs).astype(npdt)
        in_maps.append(m)
    return in_maps


def kernel(x, W1, b1, W2, b2, W3, b3, _trace=False, _mode=None):
    mode = _mode or MODE
    nb = x.shape[0] // NCORES
    nc = build_program(nb, mode)
    in_maps = _prep_host(x, W1, b1, W2, b2, W3, b3, mode)
    res = run_bass_kernel_spmd(
        nc, in_maps, core_ids=list(range(NCORES)), trace=_trace
    )
    outs = []
    for r in res.results:
        o = r["out"]  # [nb, NT, 2, JT]
        outs.append(o.transpose(0, 1, 3, 2).reshape(nb, UNITS, 2))
    full = np.concatenate(outs, 0).astype(np.float32)
    if _trace:
        kernel.last_exec_time_ns = res.exec_time_ns
        kernel.last_results = res
    return full


kernel.last_exec_time_ns = None
kernel.last_results = None
